# revision 15
# baseline (speedup 1.0000x reference)
"""Trainium2 Bass kernel: cross multi-head attention.

Problem shape: B=8, S=1024, D=1024, H=16 heads (head dim 64), fp32.

    x  = encoding_output @ Wqk + bqk          -> split into q, k per head
    v  = input_query @ Wv + bv
    out = softmax(q k^T / sqrt(64)) v @ Wo + bo

Sharding: data-parallel over batch. Core i computes batch element i
end-to-end (no collectives); weights are replicated to all 8 cores.

Per-core dataflow (matmuls run as float32r, 1 row/cycle at N=512):
  xeT, xqT : PE transpose-mode (exact fp32) -> [D, S] layouts in SBUF
  V        : natural [S, D] computed via lhsT = xqT slices, rhs = Wv, and
             stored interleaved per head pair as [v_2k | ones | v_2k+1]
             (192 cols per pair) so that each head h gets a contiguous
             128-col stationary window [v_h|ones] (even h) or [ones|v_h]
             (odd h).
  per head h:
    qkT  [128, S] : rows 0-63 = q^T, rows 64-127 = k^T
                    (lhsT = Wqk head slice, rhs = xeT)
    k_sb [64, S]  : k^T shifted to partitions 0-63 via SBUF->SBUF DMA
    scoresT       : per (sk-tile, sq-half) [128 sk, 512 sq]:
                    lhsT = k-slice, rhs = q rows
    eT            : exp(scoresT / 8) on ScalarE, PSUM -> SBUF (f32r)
    attn@v        : lhsT = the [v|ones] window -> PSUM rows hold
                    (out^T ; sums) for even h, (sums ; out^T) for odd h,
                    so the normalized result lands on the partition half
                    that its O^T chunk needs (DVE cannot cross partitions).
    normalize     : DVE reciprocal of one sums row -> DRAM round-trip DMA
                    broadcasts it across 64 partitions -> DVE multiply
                    writes the O^T chunk.
  out = lhsT O^T-slices x Wo + bo (natural [S, D] orientation) -> DRAM

Built on bacc.Bacc (not raw Bass): its compile() pass splits multi-wait
sync into event-semaphore chains (HW allows 1 wait per instruction) and
moves matmul waits onto ldweights.
"""

import sys

for _p in ("/opt/trn_rl_repo",):
    if _p not in sys.path:
        sys.path.insert(0, _p)

import numpy as np

import concourse.bass as bass
from concourse import bacc
import concourse.mybir as mybir
import concourse.tile as tile
from concourse.masks import make_identity

B, S, D = 8, 1024, 1024
H = 16
HD = D // H          # 64, head dim
P = 128              # SBUF partitions
NCH = D // P         # 8 chunks of the d/j dims
SCH = S // P         # 8 chunks of s
NH = 512             # matmul moving free-dim (fp32 max / PSUM bank)
VPAIR = 3 * HD       # 192 cols per head pair in the interleaved V layout
VROW = (H // 2) * VPAIR  # 1536
SCALE = float(1.0 / np.sqrt(np.float32(HD)))

F32 = mybir.dt.float32
F32R = mybir.dt.float32r
EXP = mybir.ActivationFunctionType.Exp


def _ap(base, off, dims):
    """Custom free-dim AP over `base` (a [128, F] AP): partition dim kept."""
    return bass.AP(
        tensor=base.tensor,
        offset=base.offset + off,
        ap=[list(base.ap[0])] + [list(d) for d in dims],
    )


def build_nc():
    nc = bacc.Bacc()

    xq_t = nc.dram_tensor("input_query", [S, D], F32, kind="ExternalInput")
    xe_t = nc.dram_tensor("encoding_output", [S, D], F32, kind="ExternalInput")
    wqk_t = nc.dram_tensor("Wqk", [D, 2 * D], F32, kind="ExternalInput")
    bqk_t = nc.dram_tensor("bqk", [2 * D], F32, kind="ExternalInput")
    wv_t = nc.dram_tensor("Wv", [D, D], F32, kind="ExternalInput")
    bv_t = nc.dram_tensor("bv", [D], F32, kind="ExternalInput")
    wo_t = nc.dram_tensor("Wo", [D, D], F32, kind="ExternalInput")
    bo_t = nc.dram_tensor("bo", [D], F32, kind="ExternalInput")
    out_t = nc.dram_tensor("out", [S, D], F32, kind="ExternalOutput")

    with tile.TileContext(nc) as tc:
        _build_kernel(tc, xq_t, xe_t, wqk_t, bqk_t, wv_t, bv_t, wo_t, bo_t, out_t)
    nc.compile()
    return nc


def _build_kernel(tc, xq_t, xe_t, wqk_t, bqk_t, wv_t, bv_t, wo_t, bo_t, out_t):
    nc = tc.nc

    with (
        tc.tile_pool(name="singles", bufs=1) as singles,
        tc.tile_pool(name="bigs", bufs=1) as bigs,
    ):
        ident = singles.tile([P, P], F32)
        make_identity(nc, ident)
        # bqk_all[p, h] = bqk[h*128 + p] — per-partition bias for qkT layout
        bqk_all = singles.tile([P, H], F32)
        nc.gpsimd.dma_start(
            out=bqk_all, in_=bass.AP(tensor=bqk_t, offset=0, ap=[[1, P], [P, H]])
        )
        bv_bc = singles.tile([P, D], F32)
        nc.gpsimd.dma_start(
            out=bv_bc, in_=bass.AP(tensor=bv_t, offset=0, ap=[[0, P], [1, D]])
        )
        bo_bc = singles.tile([P, D], F32)
        nc.gpsimd.dma_start(
            out=bo_bc, in_=bass.AP(tensor=bo_t, offset=0, ap=[[0, P], [1, D]])
        )
        ones512 = singles.tile([P, NH], F32)
        nc.vector.memset(ones512, 1.0)

        # xeT[p, dc, s] = xe[s, dc*128 + p]
        xeT = bigs.tile([P, NCH, S], F32R)
        # V_sb[p, sc, :]: per pair k cols [192k,192k+64)=v_{2k},
        # [192k+64,192k+128)=ones, [192k+128,192k+192)=v_{2k+1}
        V_sb = bigs.tile([P, SCH, VROW], F32R)
        # OT[p, jc, s] = O[s, jc*128 + p] — normalized attention output^T
        OT = bigs.tile([P, NCH, S], F32R)

        # fill the ones blocks of V_sb (DVE copy casts f32 -> f32r)
        for st in range(SCH):
            base = V_sb[:, st, :]
            nc.vector.tensor_copy(
                _ap(base, HD, [[VPAIR, NCH], [1, HD]]),
                ones512.rearrange("p (a b) -> p a b", a=NCH),
            )

        # ---------------- phase A: transposes + V projection ----------------
        with (
            tc.tile_pool(name="phA", bufs=2) as pha,
            tc.tile_pool(name="phA_xqT", bufs=1) as pxq,
            tc.tile_pool(name="psA", bufs=4, space="PSUM") as psA,
            tc.tile_pool(name="psAv", bufs=2, space="PSUM") as psAv,
        ):
            xqT = pxq.tile([P, NCH, S], F32R)
            for src, dstT in ((xq_t, xqT), (xe_t, xeT)):
                for c in range(SCH):
                    xnat = pha.tile([P, D], F32, tag="xnat", name=f"xnat{c}")
                    nc.sync.dma_start(out=xnat, in_=src[c * P : (c + 1) * P, :])
                    for dc in range(NCH):
                        pt = psA.tile([P, P], F32, tag="pt", name=f"pt{c}_{dc}")
                        nc.tensor.transpose(pt, xnat[:, dc * P : (dc + 1) * P], ident)
                        nc.vector.tensor_copy(dstT[:, dc, c * P : (c + 1) * P], pt)

            wv_r = wv_t[:].rearrange("(dc p) j -> p dc j", p=P)
            for nh in range(2):
                wv_sb = pha.tile([P, NCH, NH], F32R, tag="wv", name=f"wv{nh}")
                nc.gpsimd.dma_start(out=wv_sb, in_=wv_r[:, :, nh * NH : (nh + 1) * NH])
                for st in range(SCH):
                    pv = psAv.tile([P, NH], F32, tag="pv", name=f"pv{nh}_{st}")
                    for dc in range(NCH):
                        nc.tensor.matmul(
                            pv,
                            xqT[:, dc, st * P : (st + 1) * P],
                            wv_sb[:, dc, :],
                            start=(dc == 0),
                            stop=(dc == NCH - 1),
                        )
                    # scatter the 8 head-blocks of this half into the
                    # interleaved layout: head h -> 192*(h//2) + (h%2)*128
                    base = V_sb[:, st, :]
                    dst = _ap(
                        base,
                        nh * (NCH // 2) * VPAIR,
                        [[VPAIR, NCH // 2], [2 * HD, 2], [1, HD]],
                    )
                    nc.vector.tensor_add(
                        dst,
                        pv.rearrange("p (a b c) -> p a b c", a=NCH // 2, b=2),
                        bv_bc[:, nh * NH : (nh + 1) * NH].rearrange(
                            "p (a b c) -> p a b c", a=NCH // 2, b=2
                        ),
                    )

        # ---------------- phase B: attention heads ----------------
        with (
            tc.tile_pool(name="wqkp", bufs=2) as wqkp,
            tc.tile_pool(name="qkp", bufs=2) as qkp,
            tc.tile_pool(name="etp", bufs=1) as etp,
            tc.tile_pool(name="miscp", bufs=2) as miscp,
            tc.tile_pool(name="dramp", bufs=2, space="DRAM") as dramp,
            tc.tile_pool(name="psqk", bufs=1, space="PSUM") as psqk,
            tc.tile_pool(name="pss", bufs=2, space="PSUM") as pss,
            tc.tile_pool(name="psav", bufs=2, space="PSUM") as psav,
        ):
            wqk_r = wqk_t[:].rearrange("(dc p) j -> p dc j", p=P)
            qk_state = {}
            eT_state = {}

            def emit_qkt(h):
                wqk_sb = wqkp.tile([P, NCH, P], F32R, tag="wqk", name=f"wqk{h}")
                nc.gpsimd.dma_start(out=wqk_sb, in_=wqk_r[:, :, h * P : (h + 1) * P])
                pqk = psqk.tile([P, S], F32, tag="pqk", name=f"pqk{h}")
                for sh in range(2):
                    for dc in range(NCH):
                        nc.tensor.matmul(
                            pqk[:, sh * NH : (sh + 1) * NH],
                            wqk_sb[:, dc, :],
                            xeT[:, dc, sh * NH : (sh + 1) * NH],
                            start=(dc == 0),
                            stop=(dc == NCH - 1),
                        )
                qk_sb = qkp.tile([P, S], F32R, tag="qk", name=f"qk{h}")
                nc.vector.tensor_scalar_add(qk_sb, pqk, bqk_all[:, h : h + 1])
                k_sb = qkp.tile([64, S], F32R, tag="ksb", name=f"k{h}")
                nc.sync.dma_start(out=k_sb, in_=qk_sb[64:128, :])
                qk_state[h] = (qk_sb, k_sb)

            def emit_scores_exp(h):
                qk_sb, k_sb = qk_state[h]
                ets = []
                for skt in range(SCH):
                    et = etp.tile([P, S], F32R, tag=f"eT{skt}", name=f"eT{h}_{skt}")
                    for sh in range(2):
                        ps = pss.tile(
                            [P, NH], F32, tag="ps", name=f"ps{h}_{skt}_{sh}"
                        )
                        nc.tensor.matmul(
                            ps,
                            k_sb[:, skt * P : (skt + 1) * P],
                            qk_sb[0:64, sh * NH : (sh + 1) * NH],
                            start=True,
                            stop=True,
                        )
                        nc.scalar.activation(
                            et[:, sh * NH : (sh + 1) * NH], ps, EXP, scale=SCALE
                        )
                    ets.append(et)
                eT_state[h] = ets

            def emit_attnv(h):
                ets = eT_state.pop(h)
                qk_state.pop(h)
                pav = psav.tile([P, S], F32, tag="pav", name=f"pav{h}")
                vwin = VPAIR * (h // 2) + (h % 2) * HD
                for skc in range(SCH):
                    for sh in range(2):
                        nc.tensor.matmul(
                            pav[:, sh * NH : (sh + 1) * NH],
                            V_sb[:, skc, vwin : vwin + P],
                            ets[skc][:, sh * NH : (sh + 1) * NH],
                            start=(skc == 0),
                            stop=(skc == SCH - 1),
                        )
                # normalize: rows (0:64, 64:128) hold (out^T, sums) for even
                # h and (sums, out^T) for odd h
                rt = miscp.tile([P, S], F32, tag="recip_row", name=f"rr{h}")
                rb = miscp.tile([P, S], F32, tag="recip_bc", name=f"rb{h}")
                dsc = dramp.tile([1, S], F32, tag="dsc", name=f"dsc{h}")
                hc = h // 2
                if h % 2 == 0:
                    sums_row, out_rows = rt[64:65, :], (0, 64)
                    nc.vector.reciprocal(sums_row, pav[64:65, :])
                else:
                    sums_row, out_rows = rt[0:1, :], (64, 128)
                    nc.vector.reciprocal(sums_row, pav[0:1, :])
                nc.sync.dma_start(out=dsc, in_=sums_row)
                lo, hi = out_rows
                nc.sync.dma_start(
                    out=rb[lo:hi, :],
                    in_=bass.AP(
                        tensor=dsc.tensor, offset=dsc.offset, ap=[[0, 64], [1, S]]
                    ),
                )
                nc.vector.tensor_mul(
                    OT[lo:hi, hc, :], pav[lo:hi, :], rb[lo:hi, :]
                )

            emit_qkt(0)
            for h in range(H):
                if h + 1 < H:
                    emit_qkt(h + 1)
                emit_scores_exp(h)
                emit_attnv(h)

        # ---------------- phase C: output projection ----------------
        with (
            tc.tile_pool(name="phC", bufs=2) as phc,
            tc.tile_pool(name="psC", bufs=2, space="PSUM") as psc,
        ):
            wo_r = wo_t[:].rearrange("(jc p) n -> p jc n", p=P)
            for nh in range(2):
                wo_sb = phc.tile([P, NCH, NH], F32R, tag="wo", name=f"wo{nh}")
                nc.gpsimd.dma_start(out=wo_sb, in_=wo_r[:, :, nh * NH : (nh + 1) * NH])
                for st in range(SCH):
                    pf = psc.tile([P, NH], F32, tag="pf", name=f"pf{nh}_{st}")
                    for jc in range(NCH):
                        nc.tensor.matmul(
                            pf,
                            OT[:, jc, st * P : (st + 1) * P],
                            wo_sb[:, jc, :],
                            start=(jc == 0),
                            stop=(jc == NCH - 1),
                        )
                    fin = phc.tile([P, NH], F32, tag="fin", name=f"fin{nh}_{st}", bufs=3)
                    nc.vector.tensor_add(fin, pf, bo_bc[:, nh * NH : (nh + 1) * NH])
                    nc.sync.dma_start(
                        out=out_t[st * P : (st + 1) * P, nh * NH : (nh + 1) * NH],
                        in_=fin,
                    )


_NC_CACHE = None


def _get_nc():
    global _NC_CACHE
    if _NC_CACHE is None:
        _NC_CACHE = build_nc()
    return _NC_CACHE


def make_in_maps(inputs):
    ins = {k: np.ascontiguousarray(np.asarray(v), dtype=np.float32) for k, v in inputs.items()}
    in_maps = []
    for i in range(B):
        in_maps.append(
            dict(
                input_query=ins["input_query"][i],
                encoding_output=ins["encoding_output"][i],
                Wqk=ins["Wqk"],
                bqk=ins["bqk"],
                Wv=ins["Wv"],
                bv=ins["bv"],
                Wo=ins["Wo"],
                bo=ins["bo"],
            )
        )
    return in_maps


def kernel(**inputs):
    from concourse.bass_utils import run_bass_kernel_spmd

    nc = _get_nc()
    res = run_bass_kernel_spmd(nc, make_in_maps(inputs), list(range(B)))
    return np.stack([res.results[i]["out"] for i in range(B)], axis=0).astype(np.float32)


if __name__ == "__main__":
    nc = build_nc()
    print("built OK")


# revision 16
# speedup vs baseline: 19.1243x; 19.1243x over previous
"""Trainium2 Bass kernel: cross multi-head attention.

Problem shape: B=8, S=1024, D=1024, H=16 heads (head dim 64), fp32.

    x  = encoding_output @ Wqk + bqk          -> split into q, k per head
    v  = input_query @ Wv + bv
    out = softmax(q k^T / sqrt(64)) v @ Wo + bo

Sharding: data-parallel over batch. Core i computes batch element i
end-to-end (no collectives); weights are replicated to all 8 cores.

Per-core dataflow (matmuls run as float32r, 1 row/cycle at N=512):
  xeT, xqT : PE transpose-mode (exact fp32) -> [D, S] layouts in SBUF
  V        : natural [S, D] computed via lhsT = xqT slices, rhs = Wv, and
             stored interleaved per head pair as [v_2k | ones | v_2k+1]
             (192 cols per pair) so that each head h gets a contiguous
             128-col stationary window [v_h|ones] (even h) or [ones|v_h]
             (odd h).
  per head h:
    qkT  [128, S] : rows 0-63 = q^T, rows 64-127 = k^T
                    (lhsT = Wqk head slice, rhs = xeT)
    k_sb [64, S]  : k^T shifted to partitions 0-63 via SBUF->SBUF DMA
    scoresT       : per (sk-tile, sq-half) [128 sk, 512 sq]:
                    lhsT = k-slice, rhs = q rows
    eT            : exp(scoresT / 8) on ScalarE, PSUM -> SBUF (f32r)
    attn@v        : lhsT = the [v|ones] window -> PSUM rows hold
                    (out^T ; sums) for even h, (sums ; out^T) for odd h,
                    so the normalized result lands on the partition half
                    that its O^T chunk needs (DVE cannot cross partitions).
    normalize     : DVE reciprocal of one sums row -> DRAM round-trip DMA
                    broadcasts it across 64 partitions -> DVE multiply
                    writes the O^T chunk.
  out = lhsT O^T-slices x Wo + bo (natural [S, D] orientation) -> DRAM

Built on bacc.Bacc (not raw Bass): its compile() pass splits multi-wait
sync into event-semaphore chains (HW allows 1 wait per instruction) and
moves matmul waits onto ldweights.
"""

import sys

for _p in ("/opt/trn_rl_repo",):
    if _p not in sys.path:
        sys.path.insert(0, _p)

import numpy as np

import concourse.bass as bass
from concourse import bacc
import concourse.mybir as mybir
import concourse.tile as tile
from concourse.masks import make_identity

B, S, D = 8, 1024, 1024
H = 16
HD = D // H          # 64, head dim
P = 128              # SBUF partitions
NCH = D // P         # 8 chunks of the d/j dims
SCH = S // P         # 8 chunks of s
NH = 512             # matmul moving free-dim (fp32 max / PSUM bank)
VPAIR = 3 * HD       # 192 cols per head pair in the interleaved V layout
VROW = (H // 2) * VPAIR  # 1536
SCALE = float(1.0 / np.sqrt(np.float32(HD)))

F32 = mybir.dt.float32
F32R = mybir.dt.float32r
EXP = mybir.ActivationFunctionType.Exp


def _ap(base, off, dims):
    """Custom free-dim AP over `base` (a [128, F] AP): partition dim kept."""
    return bass.AP(
        tensor=base.tensor,
        offset=base.offset + off,
        ap=[list(base.ap[0])] + [list(d) for d in dims],
    )


def build_nc(reps=1):
    nc = bacc.Bacc()

    xq_t = nc.dram_tensor("input_query", [S, D], F32, kind="ExternalInput")
    xe_t = nc.dram_tensor("encoding_output", [S, D], F32, kind="ExternalInput")
    wqk_t = nc.dram_tensor("Wqk", [D, 2 * D], F32, kind="ExternalInput")
    bqk_t = nc.dram_tensor("bqk", [2 * D], F32, kind="ExternalInput")
    wv_t = nc.dram_tensor("Wv", [D, D], F32, kind="ExternalInput")
    bv_t = nc.dram_tensor("bv", [D], F32, kind="ExternalInput")
    wo_t = nc.dram_tensor("Wo", [D, D], F32, kind="ExternalInput")
    bo_t = nc.dram_tensor("bo", [D], F32, kind="ExternalInput")
    out_t = nc.dram_tensor("out", [S, D], F32, kind="ExternalOutput")

    with tile.TileContext(nc) as tc:
        for _rep in range(reps):
            _build_kernel(tc, xq_t, xe_t, wqk_t, bqk_t, wv_t, bv_t, wo_t, bo_t, out_t)
    nc.compile()
    return nc


def _build_kernel(tc, xq_t, xe_t, wqk_t, bqk_t, wv_t, bv_t, wo_t, bo_t, out_t):
    nc = tc.nc

    with (
        tc.tile_pool(name="singles", bufs=1) as singles,
        tc.tile_pool(name="bigs", bufs=1) as bigs,
    ):
        ident = singles.tile([P, P], F32)
        make_identity(nc, ident)
        # bqk_all[p, h] = bqk[h*128 + p] — per-partition bias for qkT layout
        bqk_all = singles.tile([P, H], F32)
        nc.gpsimd.dma_start(
            out=bqk_all, in_=bass.AP(tensor=bqk_t, offset=0, ap=[[1, P], [P, H]])
        )
        bv_bc = singles.tile([P, D], F32)
        nc.gpsimd.dma_start(
            out=bv_bc, in_=bass.AP(tensor=bv_t, offset=0, ap=[[0, P], [1, D]])
        )
        bo_bc = singles.tile([P, D], F32)
        nc.gpsimd.dma_start(
            out=bo_bc, in_=bass.AP(tensor=bo_t, offset=0, ap=[[0, P], [1, D]])
        )
        ones512 = singles.tile([P, NH], F32)
        nc.vector.memset(ones512, 1.0)

        # xeT[p, dc, s] = xe[s, dc*128 + p]
        xeT = bigs.tile([P, NCH, S], F32R)
        # V_sb[p, sc, :]: per pair k cols [192k,192k+64)=v_{2k},
        # [192k+64,192k+128)=ones, [192k+128,192k+192)=v_{2k+1}
        V_sb = bigs.tile([P, SCH, VROW], F32R)
        # OT[p, jc, s] = O[s, jc*128 + p] — normalized attention output^T
        OT = bigs.tile([P, NCH, S], F32R)

        # fill the ones blocks of V_sb (DVE copy casts f32 -> f32r)
        for st in range(SCH):
            base = V_sb[:, st, :]
            nc.vector.tensor_copy(
                _ap(base, HD, [[VPAIR, NCH], [1, HD]]),
                ones512.rearrange("p (a b) -> p a b", a=NCH),
            )

        # ---------------- phase A: transposes + V projection ----------------
        with (
            tc.tile_pool(name="phA", bufs=2) as pha,
            tc.tile_pool(name="phA_xqT", bufs=1) as pxq,
            tc.tile_pool(name="psA", bufs=4, space="PSUM") as psA,
            tc.tile_pool(name="psAv", bufs=2, space="PSUM") as psAv,
        ):
            xqT = pxq.tile([P, NCH, S], F32R)
            for src, dstT in ((xq_t, xqT), (xe_t, xeT)):
                for c in range(SCH):
                    xnat = pha.tile([P, D], F32, tag="xnat", name=f"xnat{c}")
                    nc.sync.dma_start(out=xnat, in_=src[c * P : (c + 1) * P, :])
                    for dc in range(NCH):
                        pt = psA.tile([P, P], F32, tag="pt", name=f"pt{c}_{dc}")
                        nc.tensor.transpose(pt, xnat[:, dc * P : (dc + 1) * P], ident)
                        nc.vector.tensor_copy(dstT[:, dc, c * P : (c + 1) * P], pt)

            wv_r = wv_t[:].rearrange("(dc p) j -> p dc j", p=P)
            for nh in range(2):
                wv_sb = pha.tile([P, NCH, NH], F32R, tag="wv", name=f"wv{nh}")
                nc.gpsimd.dma_start(out=wv_sb, in_=wv_r[:, :, nh * NH : (nh + 1) * NH])
                for st in range(SCH):
                    pv = psAv.tile([P, NH], F32, tag="pv", name=f"pv{nh}_{st}")
                    for dc in range(NCH):
                        nc.tensor.matmul(
                            pv,
                            xqT[:, dc, st * P : (st + 1) * P],
                            wv_sb[:, dc, :],
                            start=(dc == 0),
                            stop=(dc == NCH - 1),
                        )
                    # scatter the 8 head-blocks of this half into the
                    # interleaved layout: head h -> 192*(h//2) + (h%2)*128
                    base = V_sb[:, st, :]
                    dst = _ap(
                        base,
                        nh * (NCH // 2) * VPAIR,
                        [[VPAIR, NCH // 2], [2 * HD, 2], [1, HD]],
                    )
                    nc.vector.tensor_add(
                        dst,
                        pv.rearrange("p (a b c) -> p a b c", a=NCH // 2, b=2),
                        bv_bc[:, nh * NH : (nh + 1) * NH].rearrange(
                            "p (a b c) -> p a b c", a=NCH // 2, b=2
                        ),
                    )

        # ---------------- phase B: attention heads ----------------
        with (
            tc.tile_pool(name="wqkp", bufs=2) as wqkp,
            tc.tile_pool(name="qkp", bufs=2) as qkp,
            tc.tile_pool(name="etp", bufs=1) as etp,
            tc.tile_pool(name="miscp", bufs=2) as miscp,
            tc.tile_pool(name="dramp", bufs=2, space="DRAM") as dramp,
            tc.tile_pool(name="psqk", bufs=1, space="PSUM") as psqk,
            tc.tile_pool(name="pss", bufs=2, space="PSUM") as pss,
            tc.tile_pool(name="psav", bufs=2, space="PSUM") as psav,
        ):
            wqk_r = wqk_t[:].rearrange("(dc p) j -> p dc j", p=P)
            qk_state = {}
            eT_state = {}

            def emit_qkt(h):
                wqk_sb = wqkp.tile([P, NCH, P], F32R, tag="wqk", name=f"wqk{h}")
                nc.gpsimd.dma_start(out=wqk_sb, in_=wqk_r[:, :, h * P : (h + 1) * P])
                pqk = psqk.tile([P, S], F32, tag="pqk", name=f"pqk{h}")
                for sh in range(2):
                    for dc in range(NCH):
                        nc.tensor.matmul(
                            pqk[:, sh * NH : (sh + 1) * NH],
                            wqk_sb[:, dc, :],
                            xeT[:, dc, sh * NH : (sh + 1) * NH],
                            start=(dc == 0),
                            stop=(dc == NCH - 1),
                        )
                qk_sb = qkp.tile([P, S], F32R, tag="qk", name=f"qk{h}")
                nc.vector.tensor_scalar_add(qk_sb, pqk, bqk_all[:, h : h + 1])
                k_sb = qkp.tile([64, S], F32R, tag="ksb", name=f"k{h}")
                nc.sync.dma_start(out=k_sb, in_=qk_sb[64:128, :])
                qk_state[h] = (qk_sb, k_sb)

            def emit_scores_exp(h):
                qk_sb, k_sb = qk_state[h]
                ets = []
                for skt in range(SCH):
                    et = etp.tile([P, S], F32R, tag=f"eT{skt}", name=f"eT{h}_{skt}")
                    for sh in range(2):
                        ps = pss.tile(
                            [P, NH], F32, tag="ps", name=f"ps{h}_{skt}_{sh}"
                        )
                        nc.tensor.matmul(
                            ps,
                            k_sb[:, skt * P : (skt + 1) * P],
                            qk_sb[0:64, sh * NH : (sh + 1) * NH],
                            start=True,
                            stop=True,
                        )
                        nc.scalar.activation(
                            et[:, sh * NH : (sh + 1) * NH], ps, EXP, scale=SCALE
                        )
                    ets.append(et)
                eT_state[h] = ets

            def emit_attnv(h):
                ets = eT_state.pop(h)
                qk_state.pop(h)
                pav = psav.tile([P, S], F32, tag="pav", name=f"pav{h}")
                vwin = VPAIR * (h // 2) + (h % 2) * HD
                for skc in range(SCH):
                    for sh in range(2):
                        nc.tensor.matmul(
                            pav[:, sh * NH : (sh + 1) * NH],
                            V_sb[:, skc, vwin : vwin + P],
                            ets[skc][:, sh * NH : (sh + 1) * NH],
                            start=(skc == 0),
                            stop=(skc == SCH - 1),
                        )
                # normalize: rows (0:64, 64:128) hold (out^T, sums) for even
                # h and (sums, out^T) for odd h
                rt = miscp.tile([P, S], F32, tag="recip_row", name=f"rr{h}")
                rb = miscp.tile([P, S], F32, tag="recip_bc", name=f"rb{h}")
                dsc = dramp.tile([1, S], F32, tag="dsc", name=f"dsc{h}")
                hc = h // 2
                if h % 2 == 0:
                    sums_row, out_rows = rt[64:65, :], (0, 64)
                    nc.vector.reciprocal(sums_row, pav[64:65, :])
                else:
                    sums_row, out_rows = rt[0:1, :], (64, 128)
                    nc.vector.reciprocal(sums_row, pav[0:1, :])
                nc.sync.dma_start(out=dsc, in_=sums_row)
                lo, hi = out_rows
                nc.sync.dma_start(
                    out=rb[lo:hi, :],
                    in_=bass.AP(
                        tensor=dsc.tensor, offset=dsc.offset, ap=[[0, 64], [1, S]]
                    ),
                )
                nc.vector.tensor_mul(
                    OT[lo:hi, hc, :], pav[lo:hi, :], rb[lo:hi, :]
                )

            emit_qkt(0)
            for h in range(H):
                if h + 1 < H:
                    emit_qkt(h + 1)
                emit_scores_exp(h)
                emit_attnv(h)

        # ---------------- phase C: output projection ----------------
        with (
            tc.tile_pool(name="phC", bufs=2) as phc,
            tc.tile_pool(name="psC", bufs=2, space="PSUM") as psc,
        ):
            wo_r = wo_t[:].rearrange("(jc p) n -> p jc n", p=P)
            for nh in range(2):
                wo_sb = phc.tile([P, NCH, NH], F32R, tag="wo", name=f"wo{nh}")
                nc.gpsimd.dma_start(out=wo_sb, in_=wo_r[:, :, nh * NH : (nh + 1) * NH])
                for st in range(SCH):
                    pf = psc.tile([P, NH], F32, tag="pf", name=f"pf{nh}_{st}")
                    for jc in range(NCH):
                        nc.tensor.matmul(
                            pf,
                            OT[:, jc, st * P : (st + 1) * P],
                            wo_sb[:, jc, :],
                            start=(jc == 0),
                            stop=(jc == NCH - 1),
                        )
                    fin = phc.tile([P, NH], F32, tag="fin", name=f"fin{nh}_{st}", bufs=3)
                    nc.vector.tensor_add(fin, pf, bo_bc[:, nh * NH : (nh + 1) * NH])
                    nc.sync.dma_start(
                        out=out_t[st * P : (st + 1) * P, nh * NH : (nh + 1) * NH],
                        in_=fin,
                    )


_NC_CACHE = None


def _get_nc():
    global _NC_CACHE
    if _NC_CACHE is None:
        _NC_CACHE = build_nc()
    return _NC_CACHE


def make_in_maps(inputs):
    ins = {k: np.ascontiguousarray(np.asarray(v), dtype=np.float32) for k, v in inputs.items()}
    in_maps = []
    for i in range(B):
        in_maps.append(
            dict(
                input_query=ins["input_query"][i],
                encoding_output=ins["encoding_output"][i],
                Wqk=ins["Wqk"],
                bqk=ins["bqk"],
                Wv=ins["Wv"],
                bv=ins["bv"],
                Wo=ins["Wo"],
                bo=ins["bo"],
            )
        )
    return in_maps


def kernel(**inputs):
    from concourse.bass_utils import run_bass_kernel_spmd

    nc = _get_nc()
    res = run_bass_kernel_spmd(nc, make_in_maps(inputs), list(range(B)))
    return np.stack([res.results[i]["out"] for i in range(B)], axis=0).astype(np.float32)


if __name__ == "__main__":
    nc = build_nc()
    print("built OK")
